# revision 7
# baseline (speedup 1.0000x reference)
"""DGCNN (PyG) Trainium2 Bass kernel — data-parallel over point clouds.

Full inputs -> 8 clouds/core across 8 NeuronCores -> full [64, 512] output.

Per-cloud pipeline (P=1024 points, K=5 neighbors; feature-major "transposed"
layouts so kNN top-k and pooling reduce along the free axis):
  1. kNN #1: augmented matmul s = 2*x@x^T - |x_i|^2 - |x_j|^2 (= -dist^2) in
     f32 PSUM; DVE max8 + max_index give top-5 neighbor indices per point.
  2. EdgeConv1 via linearity: e@W1 = u_i + v_j with u = x@(W1a-W1b),
     v = x@W1b; GpSimd ap_gather collects v columns per edge; 2x 64x64 MLP
     layers as block-diag M-packed f32r matmuls; max over K.
  3. kNN #2 on x1 (f32); EdgeConv2 collapses to x2 = u2_i + max_k v2_j + b.
  4. lin1 [192->1024] f32r matmuls, per-cloud max-pool reduced from PSUM.
  5. Final MLP on pooled [8,1024] + log_softmax, all on-chip.
"""
import numpy as np

import concourse.bass as bass
import concourse.mybir as mybir
import concourse.tile as tile
from concourse import bacc
from concourse.bass_utils import run_bass_kernel_spmd
from concourse.masks import make_identity

F32 = mybir.dt.float32
F32R = mybir.dt.float32r
I16 = mybir.dt.int16
U16 = mybir.dt.uint16
AF = mybir.ActivationFunctionType
ALU = mybir.AluOpType
AX = mybir.AxisListType

P = 1024          # points per cloud
KNN = 5           # neighbors (self included)
D = 6             # input feature dim
NCLOUD = 8        # clouds per core
NCORE = 8
T = P // 128      # point tiles per cloud
EPS = 1e-5
BN_S = 1.0 / np.sqrt(1.0 + EPS)


def _emit_knn(nc, sb, ps, augL, augR, idx_all):
    """s = -d2 matmuls + top-5 per point. augL/augR [kdim, 1024] f32;
    idx_all [128, 64] int16 out (col 8t+k holds k-th best for point 128t+p)."""
    for t in range(T):
        pd2 = ps.tile([128, 1024], F32, tag="d2")
        lhsT = augL[:, t * 128:(t + 1) * 128]
        nc.tensor.matmul(pd2[:, 0:512], lhsT, augR[:, 0:512], start=True, stop=True)
        nc.tensor.matmul(pd2[:, 512:1024], lhsT, augR[:, 512:1024], start=True, stop=True)
        s_sb = sb.tile([128, 1024], F32, tag="s_sb")
        nc.scalar.copy(s_sb[:], pd2[:])
        mx = sb.tile([128, 8], F32, tag="mx")
        nc.vector.max(out=mx[:], in_=s_sb[:])
        nc.vector.max_index(out=idx_all[:, 8 * t:8 * t + 8].bitcast(U16),
                            in_max=mx[:], in_values=s_sb[:])


def _emit_idx_wrap1(nc, E1, idx1_wrap, idx_all):
    """conv1 gather idx via DRAM bounce. Edge order per half h (512 points):
    m = 20*p + 5*t' + k (p = i%128, t' = (i - 512h)//128) — point-major so
    the SBUF->DRAM extract is one DMA per half (partition dim outermost).
    idx1_wrap block b=4h+cg holds half-h list wrapped m -> (m%16, m//16)."""
    for h in range(2):
        srcv = idx_all[:, 32 * h:32 * h + 32].rearrange(
            "p (t k) -> p t k", k=8)[:, :, 0:KNN]
        dstv = E1[2560 * h:2560 * (h + 1)].rearrange(
            "(p t k) -> p t k", p=128, t=4)
        nc.sync.dma_start(dstv, srcv)
    for b in range(8):
        h = b // 4
        src = bass.AP(E1.tensor, E1.offset + 2560 * h, [[1, 16], [16, 160]])
        nc.sync.dma_start(idx1_wrap[16 * b:16 * b + 16, :], src)


def _emit_idx_wrap2(nc, E2, idx2_wrap, idx_all):
    """conv2 gather idx: full-cloud point-major edge list m = 40p + 5t + k,
    replicated into all 8 core blocks."""
    srcv = idx_all[:, :].rearrange("p (t k) -> p t k", k=8)[:, :, 0:KNN]
    dstv = E2[:].rearrange("(p t k) -> p t k", p=128, t=T)
    nc.sync.dma_start(dstv, srcv)
    for b in range(8):
        src = bass.AP(E2.tensor, E2.offset, [[1, 16], [16, 320]])
        nc.sync.dma_start(idx2_wrap[16 * b:16 * b + 16, :], src)


def build_kernel(n_clouds=NCLOUD, debug=False):
    nc = bacc.Bacc(None, target_bir_lowering=False, debug=False)
    NC = n_clouds

    pos_d = nc.dram_tensor("pos", [NC * P, D], F32, kind="ExternalInput")
    c1w1_d = nc.dram_tensor("conv1_w1", [12, 64], F32, kind="ExternalInput")
    c1w2_d = nc.dram_tensor("conv1_w2", [64, 64], F32, kind="ExternalInput")
    c1w3_d = nc.dram_tensor("conv1_w3", [64, 64], F32, kind="ExternalInput")
    c1b1_d = nc.dram_tensor("conv1_b1", [64, 1], F32, kind="ExternalInput")
    c1g1_d = nc.dram_tensor("conv1_g1", [64, 1], F32, kind="ExternalInput")
    c1be1_d = nc.dram_tensor("conv1_be1", [64, 1], F32, kind="ExternalInput")
    c1b2_d = nc.dram_tensor("conv1_b2", [64, 1], F32, kind="ExternalInput")
    c1g2_d = nc.dram_tensor("conv1_g2", [64, 1], F32, kind="ExternalInput")
    c1be2_d = nc.dram_tensor("conv1_be2", [64, 1], F32, kind="ExternalInput")
    c1b3_d = nc.dram_tensor("conv1_b3", [64, 1], F32, kind="ExternalInput")
    c2w_d = nc.dram_tensor("conv2_w", [128, 128], F32, kind="ExternalInput")
    c2b_d = nc.dram_tensor("conv2_b", [128, 1], F32, kind="ExternalInput")
    l1w_d = nc.dram_tensor("lin1_w", [192, 1024], F32, kind="ExternalInput")
    l1b_d = nc.dram_tensor("lin1_b", [128, 8], F32, kind="ExternalInput")
    mw1_d = nc.dram_tensor("mlp_w1", [1024, 512], F32, kind="ExternalInput")
    mb1_d = nc.dram_tensor("mlp_b1", [1, 512], F32, kind="ExternalInput")
    mw2_d = nc.dram_tensor("mlp_w2", [512, 256], F32, kind="ExternalInput")
    mb2_d = nc.dram_tensor("mlp_b2", [1, 256], F32, kind="ExternalInput")
    mw3_d = nc.dram_tensor("mlp_w3", [256, 512], F32, kind="ExternalInput")
    mb3_d = nc.dram_tensor("mlp_b3", [1, 512], F32, kind="ExternalInput")
    out_d = nc.dram_tensor("out", [NC, 512], F32, kind="ExternalOutput")

    dbg = {}
    if debug:
        for name, shape, dt_ in [
            ("dbg_idx1", [128, 64], I16), ("dbg_x1", [128, 512], F32),
            ("dbg_idx2", [128, 64], I16), ("dbg_x2", [128, 1024], F32),
            ("dbg_pooled", [128, 8 * NC], F32), ("dbg_g1", [128, 2560], F32),
            ("dbg_h1", [128, 2560], F32), ("dbg_wrap1", [128, 160], I16),
            ("dbg_uv", [128, 1024], F32),
        ]:
            dbg[name] = nc.dram_tensor(name, shape, dt_, kind="ExternalOutput")

    with tile.TileContext(nc) as tc:
        with (
            tc.tile_pool(name="const", bufs=1) as cw,
            tc.tile_pool(name="work", bufs=2) as wk,
            tc.tile_pool(name="big", bufs=1) as bg,
            tc.tile_pool(name="dram", bufs=2, space="DRAM") as dr,
            tc.tile_pool(name="ps", bufs=2, space="PSUM") as ps,
            tc.tile_pool(name="psm", bufs=3, space="PSUM") as psm,
        ):
            # ---------------- one-time setup ----------------
            ident = cw.tile([128, 128], F32)
            make_identity(nc, ident[:])
            ones_row = cw.tile([1, 1024], F32)
            nc.vector.memset(ones_row[:], 1.0)

            _stage_n = [0]

            def stage(shape):
                # weight staging shares the big g2 slot (setup precedes use)
                _stage_n[0] += 1
                return bg.tile(shape, F32, tag="g2", name=f"stage{_stage_n[0]}")

            # conv1 L1 stationary [6, 128] = [W1a - W1b | W1b]
            w1ab = stage([6, 128])
            nc.sync.dma_start(w1ab[:, 0:64], c1w1_d[0:6, :])
            nc.sync.dma_start(w1ab[:, 64:128], c1w1_d[6:12, :])
            w1uv = cw.tile([6, 128], F32)
            nc.vector.tensor_sub(w1uv[:, 0:64], w1ab[:, 0:64], w1ab[:, 64:128])
            nc.vector.tensor_copy(w1uv[:, 64:128], w1ab[:, 64:128])

            # conv1 L2/L3 block-diag stationaries (f32r)
            w2bd_r = cw.tile([128, 128], F32R)
            w3bd_r = cw.tile([128, 128], F32R)
            for wdst, wsrc in ((w2bd_r, c1w2_d), (w3bd_r, c1w3_d)):
                wtmp = stage([128, 128])
                nc.vector.memset(wtmp[:], 0.0)
                nc.sync.dma_start(wtmp[0:64, 0:64], wsrc[:])
                nc.sync.dma_start(wtmp[64:128, 64:128], wsrc[:])
                nc.vector.tensor_copy(wdst[:], wtmp[:])

            # conv2 stationaries: u2: W2a - W2b, v2: W2b (f32r)
            cww = stage([64, 256])
            nc.sync.dma_start(cww[:, 0:128], c2w_d[0:64, :])
            nc.sync.dma_start(cww[:, 128:256], c2w_d[64:128, :])
            w2u_r = cw.tile([64, 128], F32R)
            nc.vector.tensor_sub(w2u_r[:], cww[:, 0:128], cww[:, 128:256])
            w2v_r = cw.tile([64, 128], F32R)
            nc.vector.tensor_copy(w2v_r[:], cww[:, 128:256])

            # lin1 stationaries (f32r)
            l1a_r = cw.tile([64, 1024], F32R)
            l1t = stage([64, 1024])
            nc.sync.dma_start(l1t[:], l1w_d[0:64, :])
            nc.vector.tensor_copy(l1a_r[:], l1t[:])
            l1b_r = cw.tile([128, 1024], F32R)
            l1t2 = stage([128, 1024])
            nc.sync.dma_start(l1t2[:], l1w_d[64:192, :])
            nc.vector.tensor_copy(l1b_r[:], l1t2[:])
            l1bias = cw.tile([128, 8], F32)
            nc.sync.dma_start(l1bias[:], l1b_d[:])

            # final MLP weights (f32r) + biases
            mw1_r = cw.tile([128, 8, 512], F32R)
            mt1 = stage([128, 8, 512])
            nc.sync.dma_start(mt1[:], mw1_d.rearrange("(o p) f -> p o f", p=128))
            nc.vector.tensor_copy(mw1_r[:], mt1[:])
            mw2_r = cw.tile([128, 4, 256], F32R)
            mt2 = stage([128, 4, 256])
            nc.sync.dma_start(mt2[:], mw2_d.rearrange("(o p) f -> p o f", p=128))
            nc.vector.tensor_copy(mw2_r[:], mt2[:])
            mw3_r = cw.tile([128, 2, 512], F32R)
            mt3 = stage([128, 2, 512])
            nc.sync.dma_start(mt3[:], mw3_d.rearrange("(o p) f -> p o f", p=128))
            nc.vector.tensor_copy(mw3_r[:], mt3[:])

            def bias_r(dram, n, nm):
                tf = cw.tile([1, n], F32, name=f"{nm}_f")
                nc.sync.dma_start(tf[:], dram[:])
                tr = cw.tile([1, n], F32R, name=f"{nm}_r")
                nc.vector.tensor_copy(tr[:], tf[:])
                return tr

            mb1_r = bias_r(mb1_d, 512, "mb1")
            mb2_r = bias_r(mb2_d, 256, "mb2")
            mb3_r = bias_r(mb3_d, 512, "mb3")
            ones1_r = cw.tile([1, NC], F32R)
            nc.vector.tensor_copy(ones1_r[:], ones_row[:, 0:NC])
            ones64 = cw.tile([64, 1], F32)
            nc.vector.memset(ones64[:], 1.0)

            # BN-combined scales/biases, duplicated to both partition halves
            _dup_n = [0]

            def dup_load(dram, n=64):
                _dup_n[0] += 1
                t_ = cw.tile([128, 1], F32, name=f"dup{_dup_n[0]}")
                nc.sync.dma_start(t_[0:n, :], dram[:])
                nc.sync.dma_start(t_[n:2 * n, :], dram[:])
                return t_

            s1 = dup_load(c1g1_d)
            nc.vector.tensor_scalar_mul(s1[:], s1[:], float(BN_S))
            b1v = dup_load(c1b1_d)
            be1 = dup_load(c1be1_d)
            bc1 = cw.tile([128, 1], F32)
            nc.vector.tensor_tensor(bc1[:], b1v[:], s1[:], ALU.mult)
            nc.vector.tensor_add(bc1[:], bc1[:], be1[:])
            s2 = dup_load(c1g2_d)
            nc.vector.tensor_scalar_mul(s2[:], s2[:], float(BN_S))
            b2v = dup_load(c1b2_d)
            be2 = dup_load(c1be2_d)
            bc2 = cw.tile([128, 1], F32)
            nc.vector.tensor_tensor(bc2[:], b2v[:], s2[:], ALU.mult)
            nc.vector.tensor_add(bc2[:], bc2[:], be2[:])
            b3 = dup_load(c1b3_d)
            b2c = cw.tile([128, 1], F32)
            nc.sync.dma_start(b2c[:], c2b_d[:])

            pooled = cw.tile([128, 8, NC], F32R)

            # ---------------- per-cloud pipeline ----------------
            for c in range(NC):
                # pos load + aug prebuild [x,1,-sq | 2x,-sq,1]
                pos_sb = wk.tile([128, T, D], F32, tag="pos")
                nc.sync.dma_start(
                    pos_sb[:], pos_d[c * P:(c + 1) * P, :].rearrange(
                        "(t p) d -> p t d", p=128))
                pre = wk.tile([128, T, 16], F32, tag="pre")
                nc.scalar.copy(pre[:, :, 0:6], pos_sb[:])
                nc.vector.memset(pre[:, :, 6], 1.0)
                sqt = wk.tile([128, T, D], F32, tag="sqt")
                nc.scalar.square(sqt[:], pos_sb[:])
                nc.vector.tensor_reduce(pre[:, :, 7], sqt[:], AX.X, ALU.add,
                                        negate=True)
                nc.scalar.activation(pre[:, :, 8:14], pos_sb[:], AF.Copy, scale=2.0)
                nc.vector.tensor_copy(pre[:, :, 14], pre[:, :, 7])
                nc.vector.memset(pre[:, :, 15], 1.0)

                # transpose -> xaugL [8,1024] = [x;1;-sq], xaugR = [2x;-sq;1]
                xaugL = wk.tile([8, 1024], F32, tag="xaugL")
                xaugR = wk.tile([8, 1024], F32, tag="xaugR")
                for half in range(2):
                    ptl = psm.tile([8, 512], F32, tag="mm")
                    ptr = psm.tile([8, 512], F32, tag="mm")
                    for t4 in range(4):
                        t = half * 4 + t4
                        nc.tensor.transpose(ptl[:, 128 * t4:128 * (t4 + 1)],
                                            pre[:, t, 0:8], ident[:])
                        nc.tensor.transpose(ptr[:, 128 * t4:128 * (t4 + 1)],
                                            pre[:, t, 8:16], ident[:])
                    nc.scalar.copy(xaugL[:, 512 * half:512 * (half + 1)], ptl[:])
                    nc.scalar.copy(xaugR[:, 512 * half:512 * (half + 1)], ptr[:])

                # kNN 1
                idx_all1 = wk.tile([128, 64], I16, tag="idx_all1")
                _emit_knn(nc, wk, ps, xaugL[:], xaugR[:], idx_all1)
                idx1_wrap = wk.tile([128, 160], I16, tag="idx1_wrap")
                E1 = dr.tile([5120], I16, tag="E1")
                _emit_idx_wrap1(nc, E1[:], idx1_wrap, idx_all1[:])

                # conv1 u,v: psum rows 0-63 = uT, 64-127 = vT
                puv = ps.tile([128, 1024], F32, tag="d2")
                nc.tensor.matmul(puv[:, 0:512], w1uv[:], xaugL[0:6, 0:512],
                                 start=True, stop=True)
                nc.tensor.matmul(puv[:, 512:1024], w1uv[:], xaugL[0:6, 512:1024],
                                 start=True, stop=True)
                uv_sb = bg.tile([128, 1024], F32, tag="uvu2")
                nc.scalar.copy(uv_sb[:], puv[:])
                if debug and c == 0:
                    nc.sync.dma_start(dbg["dbg_uv"][:], uv_sb[:])
                    nc.sync.dma_start(dbg["dbg_idx1"][:], idx_all1[:])
                    nc.sync.dma_start(dbg["dbg_wrap1"][:], idx1_wrap[:])
                vv = bg.tile([128, 1024], F32, tag="vx")
                nc.sync.dma_start(vv[0:64, :], uv_sb[64:128, :])
                nc.sync.dma_start(vv[64:128, :], uv_sb[64:128, :])
                udup = wk.tile([128, 512], F32, tag="udup")
                nc.sync.dma_start(udup[0:64, :], uv_sb[0:64, 0:512])
                nc.sync.dma_start(udup[64:128, :], uv_sb[0:64, 512:1024])

                # gather v per edge: layout [128, K, 512] k-major, halves on
                # partition halves
                g1 = bg.tile([128, KNN * 512], F32, tag="g1")
                nc.gpsimd.ap_gather(g1[:], vv[:], idx1_wrap[:], channels=128,
                                    num_elems=1024, d=1, num_idxs=KNN * 512)
                if debug and c == 0:
                    nc.sync.dma_start(dbg["dbg_g1"][:], g1[:])
                g1v = g1[:].rearrange("f (p t k) -> f p t k", t=4, k=KNN)
                urep = udup[:].rearrange("f (t p) -> f p t", p=128).unsqueeze(
                    3).to_broadcast([128, 128, 4, KNN])
                nc.vector.tensor_tensor(g1v, g1v, urep, ALU.add)
                h1 = bg.tile([128, KNN * 512], F32R, tag="h1")
                nc.scalar.activation(h1[:], g1[:], AF.Relu, scale=s1[:], bias=bc1[:])
                if debug and c == 0:
                    nc.sync.dma_start(dbg["dbg_h1"][:], h1[:].bitcast(F32))

                # L2 + bnrelu, L3 + copy (block-diag stationaries)
                h2 = bg.tile([128, KNN * 512], F32R, tag="h2")
                for ch in range(KNN):
                    pl2 = psm.tile([128, 512], F32, tag="mm")
                    nc.tensor.matmul(pl2[:], w2bd_r[:], h1[:, 512 * ch:512 * (ch + 1)],
                                     start=True, stop=True)
                    nc.scalar.activation(h2[:, 512 * ch:512 * (ch + 1)], pl2[:],
                                         AF.Relu, scale=s2[:], bias=bc2[:])
                h3 = bg.tile([128, KNN * 512], F32, tag="g1")
                for ch in range(KNN):
                    pl3 = psm.tile([128, 512], F32, tag="mm")
                    nc.tensor.matmul(pl3[:], w3bd_r[:], h2[:, 512 * ch:512 * (ch + 1)],
                                     start=True, stop=True)
                    nc.scalar.copy(h3[:, 512 * ch:512 * (ch + 1)], pl3[:])

                # max over K + b3 -> x1 [128, 512] (point halves on partitions)
                h3v = h3[:].rearrange("f (g k) -> f g k", k=KNN)
                x1m = wk.tile([128, 512], F32, tag="x1m")
                nc.vector.tensor_tensor(x1m[:], h3v[:, :, 0], h3v[:, :, 1], ALU.max)
                nc.vector.tensor_tensor(x1m[:], x1m[:], h3v[:, :, 2], ALU.max)
                nc.vector.tensor_tensor(x1m[:], x1m[:], h3v[:, :, 3], ALU.max)
                nc.vector.tensor_tensor(x1m[:], x1m[:], h3v[:, :, 4], ALU.max)
                x1_sb = wk.tile([128, 512], F32, tag="x1_sb")
                nc.vector.tensor_scalar_add(x1_sb[:], x1m[:], b3[:])
                if debug and c == 0:
                    nc.sync.dma_start(dbg["dbg_x1"][:], x1_sb[:])

                # x1aug [66, 1024]: rows 0-63 x1T, 64 ones, 65 -sq2
                x1aug = wk.tile([66, 1024], F32, tag="x1aug")
                nc.vector.tensor_copy(x1aug[0:64, 0:512], x1_sb[0:64, :])
                nc.sync.dma_start(x1aug[0:64, 512:1024], x1_sb[64:128, :])
                nc.sync.dma_start(x1aug[64:65, :], ones_row[:])
                x1sq = bg.tile([64, 1024], F32, tag="vx")
                nc.scalar.square(x1sq[:], x1aug[0:64, :])
                nsq2 = wk.tile([1, 1024], F32, tag="nsq2")
                for ch in range(2):
                    psq = psm.tile([1, 512], F32, tag="mm")
                    nc.tensor.matmul(psq[:], ones64[:],
                                     x1sq[:, 512 * ch:512 * (ch + 1)],
                                     start=True, stop=True)
                    nc.scalar.activation(nsq2[:, 512 * ch:512 * (ch + 1)], psq[:],
                                         AF.Copy, scale=-1.0)
                nc.sync.dma_start(x1aug[65:66, :], nsq2[:])
                rhs2 = wk.tile([66, 1024], F32, tag="rhs2")
                nc.scalar.activation(rhs2[0:64, :], x1aug[0:64, :], AF.Copy,
                                     scale=2.0)
                nc.sync.dma_start(rhs2[64:65, :], nsq2[:])
                nc.sync.dma_start(rhs2[65:66, :], ones_row[:])
                x1r = wk.tile([64, 1024], F32R, tag="x1r")
                nc.scalar.copy(x1r[:], x1aug[0:64, :])

                # kNN 2
                idx_all2 = wk.tile([128, 64], I16, tag="idx_all2")
                _emit_knn(nc, wk, ps, x1aug[:], rhs2[:], idx_all2)
                idx2_wrap = wk.tile([128, 320], I16, tag="idx2_wrap")
                E2 = dr.tile([5120], I16, tag="E2")
                _emit_idx_wrap2(nc, E2[:], idx2_wrap, idx_all2[:])
                if debug and c == 0:
                    nc.sync.dma_start(dbg["dbg_idx2"][:], idx_all2[:])

                # conv2: u2, v2 [128, 1024]
                pu2 = ps.tile([128, 1024], F32, tag="d2")
                nc.tensor.matmul(pu2[:, 0:512], w2u_r[:], x1r[:, 0:512],
                                 start=True, stop=True)
                nc.tensor.matmul(pu2[:, 512:1024], w2u_r[:], x1r[:, 512:1024],
                                 start=True, stop=True)
                u2_sb = bg.tile([128, 1024], F32, tag="uvu2")
                nc.scalar.copy(u2_sb[:], pu2[:])
                pv2 = ps.tile([128, 1024], F32, tag="d2")
                nc.tensor.matmul(pv2[:, 0:512], w2v_r[:], x1r[:, 0:512],
                                 start=True, stop=True)
                nc.tensor.matmul(pv2[:, 512:1024], w2v_r[:], x1r[:, 512:1024],
                                 start=True, stop=True)
                v2_sb = bg.tile([128, 1024], F32, tag="v2_sb")
                nc.scalar.copy(v2_sb[:], pv2[:])

                g2 = bg.tile([128, KNN * 1024], F32, tag="g2")
                nc.gpsimd.ap_gather(g2[:], v2_sb[:], idx2_wrap[:], channels=128,
                                    num_elems=1024, d=1, num_idxs=KNN * 1024)
                g2v = g2[:].rearrange("f (g k) -> f g k", k=KNN)
                x2m = bg.tile([128, 1024], F32, tag="vx")
                nc.vector.tensor_tensor(x2m[:], g2v[:, :, 0], g2v[:, :, 1], ALU.max)
                nc.vector.tensor_tensor(x2m[:], x2m[:], g2v[:, :, 2], ALU.max)
                nc.vector.tensor_tensor(x2m[:], x2m[:], g2v[:, :, 3], ALU.max)
                nc.vector.tensor_tensor(x2m[:], x2m[:], g2v[:, :, 4], ALU.max)
                # x2m col g = 8p + t holds point q = 128t + p; unscramble on write
                x2_sb = bg.tile([128, 1024], F32R, tag="x2_sb")
                nc.vector.scalar_tensor_tensor(
                    x2_sb[:].rearrange("f (t p) -> f p t", p=128),
                    x2m[:].rearrange("f (p t) -> f p t", t=T),
                    b2c[:],
                    u2_sb[:].rearrange("f (t p) -> f p t", p=128),
                    ALU.add, ALU.add)
                if debug and c == 0:
                    nc.sync.dma_start(dbg["dbg_x2"][:], x2_sb[:].bitcast(F32))

                # lin1 + max-pool into pooled[:, mt, c]
                for mt in range(8):
                    red = []
                    for ch in range(2):
                        pl = psm.tile([128, 512], F32, tag="mm")
                        nc.tensor.matmul(pl[:], l1a_r[:, 128 * mt:128 * (mt + 1)],
                                         x1r[:, 512 * ch:512 * (ch + 1)],
                                         start=True, stop=False)
                        nc.tensor.matmul(pl[:], l1b_r[:, 128 * mt:128 * (mt + 1)],
                                         x2_sb[:, 512 * ch:512 * (ch + 1)],
                                         start=False, stop=True)
                        r = wk.tile([128, 1], F32, tag=f"red{ch}", name=f"red{ch}")
                        nc.vector.tensor_reduce(r[:], pl[:], AX.X, ALU.max)
                        red.append(r)
                    rm = wk.tile([128, 1], F32, tag="rm")
                    nc.vector.tensor_tensor(rm[:], red[0][:], red[1][:], ALU.max)
                    nc.vector.tensor_scalar_add(pooled[:, mt, c:c + 1], rm[:],
                                                l1bias[:, mt:mt + 1])

            # ---------------- final MLP + log_softmax ----------------
            if debug:
                nc.sync.dma_start(
                    dbg["dbg_pooled"][:],
                    pooled[:].bitcast(F32).rearrange("p m c -> p (m c)"))
            pm1 = psm.tile([NC, 512], F32, tag="mm")
            for kt in range(8):
                nc.tensor.matmul(pm1[:], pooled[:, kt, :], mw1_r[:, kt, :],
                                 start=(kt == 0), stop=False)
            nc.tensor.matmul(pm1[:], ones1_r[:], mb1_r[:], start=False, stop=True)
            hm1 = cw.tile([NC, 512], F32)
            nc.scalar.activation(hm1[:], pm1[:], AF.Relu)
            h1T = cw.tile([128, 4, NC], F32R)
            for q in range(4):
                pt = psm.tile([128, NC], F32, tag="mm")
                nc.tensor.transpose(pt[:], hm1[:, 128 * q:128 * (q + 1)],
                                    ident[0:NC, 0:NC])
                nc.scalar.copy(h1T[:, q, :], pt[:])
            pm2 = psm.tile([NC, 256], F32, tag="mm")
            for kt in range(4):
                nc.tensor.matmul(pm2[:], h1T[:, kt, :], mw2_r[:, kt, :],
                                 start=(kt == 0), stop=False)
            nc.tensor.matmul(pm2[:], ones1_r[:], mb2_r[:], start=False, stop=True)
            hm2 = cw.tile([NC, 256], F32)
            nc.scalar.activation(hm2[:], pm2[:], AF.Relu)
            h2T = cw.tile([128, 2, NC], F32R)
            for q in range(2):
                pt = psm.tile([128, NC], F32, tag="mm")
                nc.tensor.transpose(pt[:], hm2[:, 128 * q:128 * (q + 1)],
                                    ident[0:NC, 0:NC])
                nc.scalar.copy(h2T[:, q, :], pt[:])
            pm3 = psm.tile([NC, 512], F32, tag="mm")
            for kt in range(2):
                nc.tensor.matmul(pm3[:], h2T[:, kt, :], mw3_r[:, kt, :],
                                 start=(kt == 0), stop=False)
            nc.tensor.matmul(pm3[:], ones1_r[:], mb3_r[:], start=False, stop=True)

            rmax = cw.tile([NC, 1], F32)
            nc.vector.tensor_reduce(rmax[:], pm3[:], AX.X, ALU.max)
            nrmax = cw.tile([NC, 1], F32)
            nc.vector.tensor_scalar_mul(nrmax[:], rmax[:], -1.0)
            expv = cw.tile([NC, 512], F32)
            sumexp = cw.tile([NC, 1], F32)
            nc.scalar.activation(expv[:], pm3[:], AF.Exp, bias=nrmax[:],
                                 accum_out=sumexp[:])
            lse = cw.tile([NC, 1], F32)
            nc.scalar.activation(lse[:], sumexp[:], AF.Ln)
            out_sb = cw.tile([NC, 512], F32)
            nc.vector.tensor_scalar(out_sb[:], pm3[:], rmax[:], lse[:],
                                    ALU.subtract, ALU.subtract)
            nc.sync.dma_start(out_d[:], out_sb[:])

    nc.compile()
    return nc


_CACHED = {}


def _get_kernel(n_clouds=NCLOUD, debug=False):
    key = (n_clouds, debug)
    if key not in _CACHED:
        _CACHED[key] = build_kernel(n_clouds, debug)
    return _CACHED[key]


def make_in_maps(inputs, n_cores=NCORE):
    """Slice full inputs into per-core input maps (reshaping 1-D vectors)."""
    pos = np.ascontiguousarray(np.asarray(inputs["pos"], dtype=np.float32))
    w = {k: np.ascontiguousarray(np.asarray(v), dtype=np.float32)
         for k, v in inputs.items()
         if k not in ("pos", "batch", "num_graphs")}
    common = {
        "conv1_w1": w["conv1_w1"], "conv1_w2": w["conv1_w2"],
        "conv1_w3": w["conv1_w3"],
        "conv1_b1": w["conv1_b1"].reshape(64, 1),
        "conv1_g1": w["conv1_g1"].reshape(64, 1),
        "conv1_be1": w["conv1_be1"].reshape(64, 1),
        "conv1_b2": w["conv1_b2"].reshape(64, 1),
        "conv1_g2": w["conv1_g2"].reshape(64, 1),
        "conv1_be2": w["conv1_be2"].reshape(64, 1),
        "conv1_b3": w["conv1_b3"].reshape(64, 1),
        "conv2_w": w["conv2_w"], "conv2_b": w["conv2_b"].reshape(128, 1),
        "lin1_w": w["lin1_w"],
        "lin1_b": np.ascontiguousarray(w["lin1_b"].reshape(8, 128).T),
        "mlp_w1": w["mlp_w1"], "mlp_b1": w["mlp_b1"].reshape(1, 512),
        "mlp_w2": w["mlp_w2"], "mlp_b2": w["mlp_b2"].reshape(1, 256),
        "mlp_w3": w["mlp_w3"], "mlp_b3": w["mlp_b3"].reshape(1, 512),
    }
    in_maps = []
    rows = NCLOUD * P
    for core in range(n_cores):
        m = dict(common)
        m["pos"] = pos[core * rows:(core + 1) * rows]
        in_maps.append(m)
    return in_maps


def kernel(**inputs) -> np.ndarray:
    nc = _get_kernel()
    in_maps = make_in_maps(inputs)
    res = run_bass_kernel_spmd(nc, in_maps, core_ids=list(range(NCORE)))
    return np.concatenate([r["out"] for r in res.results], axis=0)
